# revision 28
# baseline (speedup 1.0000x reference)
"""AtomEmbedding kernel for 8 TRN2 NeuronCores.

Reference semantics: rank-remap of atom types through the sorted unique
values present in the batch, then embedding lookup:
    uniq = unique(atom_types)  (padded sorted)
    out[b, a] = embedding[searchsorted(uniq, atom_types[b, a])]

Device strategy (data-parallel over n_batch): the lookup is computed as a
one-hot matmul on the TensorEngine (a per-row DMA gather is descriptor-
generation-bound on the Q7 cores at ~7 ns/row — measured 542 us/core):

  per 512-atom chunk k:
    PE   : rep[v, q] = ones[1,100].T @ idx_row[1,512]     (replicate idx)
    ACT  : sq[v, q]  = copy(rep)  bf16                    (drain PSUM)
    DVE  : oh[v, q]  = (sq == iota_v)                     (fp32 one-hot)
    PE   : out[p, d] = oh[:, s*128:].T @ table[100, 64]   (exact fp32)
    ACT/DVE alternate PSUM->SBUF output copies; sync writes 4KB/partition.

Raw-bass engine blocks with standalone wait_ge (the neuronxcc walrus in
this toolchain cannot encode multi-wait sync on one instruction).

Self-contained: shapes hardcoded, no sibling imports.
"""

import sys

if "/opt/trn_rl_repo" not in sys.path:
    sys.path.insert(0, "/opt/trn_rl_repo")

import numpy as np

N_BATCH = 9000
ATOMS_PER_MOL = 64
EMBED_DIM = 64
NUM_TYPES = 100
N_CORES = 8

ROWS_PER_CORE = N_BATCH * ATOMS_PER_MOL // N_CORES  # 72000
T_SLOTS = 576  # padded slots per partition
PAD_ROWS = 128 * T_SLOTS  # 73728
N_CHUNKS = T_SLOTS // 4  # 144 chunks of 512 atoms
N_GROUPS = N_CHUNKS // 4  # 36 writeback groups of 16 slots
IDX_LEN = N_CHUNKS * 512 + 128  # feed + trailing ones row

_CACHE = {}


def _build_graph():
    import concourse.bass as bass
    import concourse.mybir as mybir

    f32 = mybir.dt.float32
    bf16 = mybir.dt.bfloat16
    AF = mybir.ActivationFunctionType
    OP = mybir.AluOpType

    nc = bass.Bass()

    idx_d = nc.declare_dram_parameter("idx", [1, IDX_LEN], bf16, isOutput=False)
    const_d = nc.declare_dram_parameter("const", [128, 65], f32, isOutput=False)
    out_d = nc.declare_dram_parameter("out", [PAD_ROWS, EMBED_DIM], f32, isOutput=True)

    with (
        nc.sbuf_tensor([1, IDX_LEN], bf16) as idx_sb,
        nc.sbuf_tensor([128, 65], f32) as const_sb,
        nc.sbuf_tensor([100, 3 * 512], bf16) as sq_sb,
        nc.sbuf_tensor([100, 3 * 512], f32) as oh_sb,
        nc.sbuf_tensor([128, 2 * 1024], f32) as outb_sb,
        nc.psum_tensor([100, 512], f32) as rep0,
        nc.psum_tensor([100, 512], f32) as rep1,
        nc.psum_tensor([128, 512], f32) as pout0,
        nc.psum_tensor([128, 512], f32) as pout1,
        nc.semaphore("in0") as in0,
        nc.semaphore("in1") as in1,
        nc.semaphore("rep_rdy") as rep_rdy,
        nc.semaphore("sq_rdy") as sq_rdy,
        nc.semaphore("oh_rdy") as oh_rdy,
        nc.semaphore("pout_rdy") as pout_rdy,
        nc.semaphore("wbA") as wbA,
        nc.semaphore("wbB") as wbB,
        nc.semaphore("wbd0") as wbd0,
        nc.semaphore("wbd1") as wbd1,
        nc.Block() as block,
    ):
        rep = [rep0, rep1]
        pout = [pout0, pout1]
        wbd = [wbd0, wbd1]
        ones_ap = idx_sb[0:1, N_CHUNKS * 512 : N_CHUNKS * 512 + 100]
        iota_ap = const_sb[:100, 0:1]
        table_ap = const_sb[:100, 1:65]
        out_flat = out_d[:].rearrange("(p t) d -> p (t d)", p=128)

        def sqv(k):
            return sq_sb[:, (k % 3) * 512 : (k % 3) * 512 + 512]

        def ohv(k):
            return oh_sb[:, (k % 3) * 512 : (k % 3) * 512 + 512]

        def outv(k4, ci):
            b = k4 % 2
            return outb_sb[:, b * 1024 + ci * 256 : b * 1024 + ci * 256 + 256]

        def copy_wait(eng, k):
            # wait until the pout-copy of chunk k is done (ACT: even, DVE: odd)
            if k % 2 == 0:
                eng.wait_ge(wbA, k // 2 + 1)
            else:
                eng.wait_ge(wbB, (k - 1) // 2 + 1)

        @block.tensor
        def _(te):
            te.wait_ge(in0, 16)
            te.wait_ge(in1, 16)
            for j in range(N_CHUNKS + 2):
                if j < N_CHUNKS:
                    if j >= 2:
                        te.wait_ge(sq_rdy, j - 1)  # ACT freed rep[j%2]
                    te.matmul(
                        out=rep[j % 2][:],
                        lhsT=ones_ap,
                        rhs=idx_sb[0:1, j * 512 : (j + 1) * 512],
                        start=True,
                        stop=True,
                    ).then_inc(rep_rdy, 1)
                if j >= 2:
                    k = j - 2
                    if k >= 2:
                        copy_wait(te, k - 2)  # copy freed pout[k%2]
                    te.wait_ge(oh_rdy, k + 1)
                    for s in range(4):
                        mm = te.matmul(
                            out=pout[k % 2][:, s * 64 : (s + 1) * 64],
                            lhsT=ohv(k)[:, s * 128 : (s + 1) * 128],
                            rhs=table_ap,
                            start=True,
                            stop=True,
                        )
                        if s == 3:
                            mm.then_inc(pout_rdy, 1)

        @block.scalar
        def _(act):
            for k in range(N_CHUNKS + 2):
                if k < N_CHUNKS:
                    act.wait_ge(rep_rdy, k + 1)
                    if k >= 3:
                        act.wait_ge(oh_rdy, k - 2)  # DVE freed sq[k%3]
                    act.activation(out=sqv(k), in_=rep[k % 2][:], func=AF.Copy).then_inc(
                        sq_rdy, 1
                    )
                kc = k - 2
                if kc >= 0 and kc % 2 == 0:
                    act.wait_ge(pout_rdy, kc + 1)
                    g = kc // 4
                    if g >= 2:
                        act.wait_ge(wbd[g % 2], (g // 2) * 16)
                    act.activation(
                        out=outv(g, kc % 4), in_=pout[kc % 2][:, :256], func=AF.Copy
                    ).then_inc(wbA, 1)

        @block.vector
        def _(dve):
            dve.wait_ge(in1, 16)
            for k in range(N_CHUNKS + 2):
                if k < N_CHUNKS:
                    dve.wait_ge(sq_rdy, k + 1)
                    if k >= 3:
                        dve.wait_ge(pout_rdy, k - 2)  # PE freed oh[k%3]
                    dve.tensor_scalar(
                        out=ohv(k),
                        in0=sqv(k),
                        scalar1=iota_ap,
                        scalar2=None,
                        op0=OP.is_equal,
                    ).then_inc(oh_rdy, 1)
                kc = k - 2
                if kc >= 0 and kc % 2 == 1:
                    dve.wait_ge(pout_rdy, kc + 1)
                    g = kc // 4
                    if g >= 2:
                        dve.wait_ge(wbd[g % 2], (g // 2) * 16)
                    dve.tensor_copy(
                        out=outv(g, kc % 4), in_=pout[kc % 2][:, :256]
                    ).then_inc(wbB, 1)

        @block.sync
        def _(sync):
            sync.dma_start(out=idx_sb[:], in_=idx_d[:]).then_inc(in0, 16)
            sync.dma_start(out=const_sb[:], in_=const_d[:]).then_inc(in1, 16)
            for g in range(N_GROUPS):
                sync.wait_ge(wbA, 2 * (g + 1))
                sync.wait_ge(wbB, 2 * (g + 1))
                if g >= 2:
                    sync.wait_ge(wbd[g % 2], (g // 2) * 16)
                sync.dma_start(
                    out=out_flat[:, g * 1024 : (g + 1) * 1024],
                    in_=outb_sb[:, (g % 2) * 1024 : (g % 2) * 1024 + 1024],
                ).then_inc(wbd[g % 2], 16)
            sync.wait_ge(wbd0, (N_GROUPS // 2) * 16)
            sync.wait_ge(wbd1, (N_GROUPS // 2) * 16)

    return nc


def _prep_in_maps(atom_types, embedding):
    import ml_dtypes

    at = np.asarray(atom_types).astype(np.int32).reshape(-1)
    emb = np.asarray(embedding).astype(np.float32)

    # rank-remap: table2[x] = embedding[rank(x)] where rank(x) counts the
    # distinct values < x present anywhere in the batch (identity when all
    # NUM_TYPES values appear).
    present = np.zeros(NUM_TYPES, dtype=bool)
    present[at] = True
    rank = np.cumsum(present) - present
    table2 = emb[np.minimum(rank, NUM_TYPES - 1)].astype(np.float32)
    table2[~present] = 0.0

    const_in = np.zeros((128, 65), np.float32)
    const_in[:, 0] = np.arange(128)
    const_in[:100, 1:65] = table2

    in_maps = []
    for c in range(N_CORES):
        shard = at[c * ROWS_PER_CORE : (c + 1) * ROWS_PER_CORE]
        sp = np.concatenate(
            [shard, np.full(PAD_ROWS - ROWS_PER_CORE, shard[0], np.int32)]
        )
        # atom (p, t) = sp[p*T + t]; chunk k covers slots 4k..4k+3;
        # within-chunk position q = s*128 + p.
        grid = sp.reshape(128, T_SLOTS)  # [p, t]
        feed = grid.reshape(128, N_CHUNKS, 4).transpose(1, 2, 0)  # [k, s, p]
        idx_in = np.ones((1, IDX_LEN), np.float32)
        idx_in[0, : N_CHUNKS * 512] = feed.reshape(-1)
        in_maps.append(
            {
                "idx": idx_in.astype(ml_dtypes.bfloat16),
                "const": const_in,
            }
        )
    return in_maps


def run(atom_types, embedding, trace=False):
    from concourse.bass_utils import run_bass_kernel_spmd

    if "nc" not in _CACHE:
        _CACHE["nc"] = _build_graph()
    nc = _CACHE["nc"]

    in_maps = _prep_in_maps(atom_types, embedding)
    res = run_bass_kernel_spmd(
        nc, in_maps, core_ids=list(range(N_CORES)), trace=trace
    )
    shards = [r["out"][:ROWS_PER_CORE] for r in res.results]
    full = np.concatenate(shards, axis=0).reshape(N_BATCH, ATOMS_PER_MOL, EMBED_DIM)
    return np.ascontiguousarray(full, dtype=np.float32), res


def kernel(atom_types, embedding):
    out, _ = run(atom_types, embedding, trace=False)
    return out


# revision 29
# speedup vs baseline: 1.3372x; 1.3372x over previous
"""AtomEmbedding kernel for 8 TRN2 NeuronCores.

Reference semantics: rank-remap of atom types through the sorted unique
values present in the batch, then embedding lookup:
    uniq = unique(atom_types)  (padded sorted)
    out[b, a] = embedding[searchsorted(uniq, atom_types[b, a])]

Device strategy (data-parallel over n_batch): the lookup is computed as a
one-hot matmul on the TensorEngine (a per-row DMA gather is descriptor-
generation-bound on the Q7 cores at ~7 ns/row — measured 542 us/core):

  per 512-atom chunk k:
    ACT  : issues a broadcast HWDGE DMA replicating the chunk's bf16
           index row from DRAM onto 100 SBUF partitions (0-stride AP)
    DVE  : oh[v, q] = (rep == iota_v)        bf16 one-hot, SBUF mode
    PE   : out[p, d] = oh[:, s*128:].T @ table_bf16[100, 64]
    ACT/DVE alternate PSUM->SBUF output copies; sync writes 4KB/partition.

Raw-bass engine blocks with standalone wait_ge (the neuronxcc walrus in
this toolchain cannot encode multi-wait sync on one instruction).

Self-contained: shapes hardcoded, no sibling imports.
"""

import sys

if "/opt/trn_rl_repo" not in sys.path:
    sys.path.insert(0, "/opt/trn_rl_repo")

import numpy as np

N_BATCH = 9000
ATOMS_PER_MOL = 64
EMBED_DIM = 64
NUM_TYPES = 100
N_CORES = 8

ROWS_PER_CORE = N_BATCH * ATOMS_PER_MOL // N_CORES  # 72000
T_SLOTS = 576  # padded slots per partition
PAD_ROWS = 128 * T_SLOTS  # 73728
N_CHUNKS = T_SLOTS // 4  # 144 chunks of 512 atoms
N_GROUPS = N_CHUNKS // 4  # 36 writeback groups of 16 slots

_CACHE = {}


def _build_graph():
    import concourse.bass as bass
    import concourse.mybir as mybir

    f32 = mybir.dt.float32
    bf16 = mybir.dt.bfloat16
    AF = mybir.ActivationFunctionType
    OP = mybir.AluOpType

    nc = bass.Bass()

    idx_d = nc.declare_dram_parameter("idx", [1, N_CHUNKS * 512], bf16, isOutput=False)
    iota_d = nc.declare_dram_parameter("iota", [128, 1], f32, isOutput=False)
    tbl_d = nc.declare_dram_parameter("tbl", [128, 64], bf16, isOutput=False)
    out_d = nc.declare_dram_parameter("out", [PAD_ROWS, EMBED_DIM], f32, isOutput=True)

    with (
        nc.sbuf_tensor([128, 1], f32) as iota_sb,
        nc.sbuf_tensor([128, 64], bf16) as tbl_sb,
        nc.sbuf_tensor([100, 3 * 512], bf16) as rep_sb,
        nc.sbuf_tensor([100, 3 * 512], bf16) as oh_sb,
        nc.sbuf_tensor([128, 2 * 1024], f32) as outb_sb,
        nc.psum_tensor([128, 512], f32) as pout0,
        nc.psum_tensor([128, 512], f32) as pout1,
        nc.semaphore("in1") as in1,
        nc.semaphore("in2") as in2,
        nc.semaphore("rd0") as rd0,
        nc.semaphore("rd1") as rd1,
        nc.semaphore("rd2") as rd2,
        nc.semaphore("oh_rdy") as oh_rdy,
        nc.semaphore("pout_rdy") as pout_rdy,
        nc.semaphore("wbA") as wbA,
        nc.semaphore("wbB") as wbB,
        nc.semaphore("wbd0") as wbd0,
        nc.semaphore("wbd1") as wbd1,
        nc.Block() as block,
    ):
        pout = [pout0, pout1]
        rdma = [rd0, rd1, rd2]
        wbd = [wbd0, wbd1]
        iota_ap = iota_sb[:100, 0:1]
        out_flat = out_d[:].rearrange("(p t) d -> p (t d)", p=128)

        def repv(k):
            return rep_sb[:, (k % 3) * 512 : (k % 3) * 512 + 512]

        def ohv(k):
            return oh_sb[:, (k % 3) * 512 : (k % 3) * 512 + 512]

        def outv(g, ci):
            b = g % 2
            return outb_sb[:, b * 1024 + ci * 256 : b * 1024 + ci * 256 + 256]

        def copy_wait(eng, k):
            # wait until the pout-copy of chunk k is done (ACT: even, DVE: odd)
            if k % 2 == 0:
                eng.wait_ge(wbA, k // 2 + 1)
            else:
                eng.wait_ge(wbB, (k - 1) // 2 + 1)

        def bcast_src(k):
            ap = idx_d[:]
            return bass.AP(ap.tensor, k * 512, [[0, 100], [1, 512]])

        @block.scalar
        def _(act):
            for k in range(N_CHUNKS + 2):
                if k < N_CHUNKS:
                    if k >= 3:
                        act.wait_ge(oh_rdy, k - 2)  # DVE freed rep[k%3]
                        act.wait_ge(rdma[k % 3], (k // 3) * 16)
                    act.dma_start(out=repv(k), in_=bcast_src(k)).then_inc(
                        rdma[k % 3], 16
                    )
                kc = k - 2
                if kc >= 0 and kc % 2 == 0:
                    act.wait_ge(pout_rdy, kc + 1)
                    g = kc // 4
                    if g >= 2:
                        act.wait_ge(wbd[g % 2], (g // 2) * 16)
                    act.activation(
                        out=outv(g, kc % 4), in_=pout[kc % 2][:, :256], func=AF.Copy
                    ).then_inc(wbA, 1)

        @block.vector
        def _(dve):
            dve.wait_ge(in1, 16)
            for k in range(N_CHUNKS + 2):
                if k < N_CHUNKS:
                    dve.wait_ge(rdma[k % 3], (k // 3 + 1) * 16)
                    if k >= 3:
                        dve.wait_ge(pout_rdy, k - 2)  # PE freed oh[k%3]
                    dve.tensor_scalar(
                        out=ohv(k),
                        in0=repv(k),
                        scalar1=iota_ap,
                        scalar2=None,
                        op0=OP.is_equal,
                    ).then_inc(oh_rdy, 1)
                kc = k - 2
                if kc >= 0 and kc % 2 == 1:
                    dve.wait_ge(pout_rdy, kc + 1)
                    g = kc // 4
                    if g >= 2:
                        dve.wait_ge(wbd[g % 2], (g // 2) * 16)
                    dve.tensor_copy(
                        out=outv(g, kc % 4), in_=pout[kc % 2][:, :256]
                    ).then_inc(wbB, 1)

        @block.tensor
        def _(te):
            te.wait_ge(in2, 16)
            for k in range(N_CHUNKS):
                if k >= 2:
                    copy_wait(te, k - 2)  # copy freed pout[k%2]
                te.wait_ge(oh_rdy, k + 1)
                for s in range(4):
                    mm = te.matmul(
                        out=pout[k % 2][:, s * 64 : (s + 1) * 64],
                        lhsT=ohv(k)[:, s * 128 : (s + 1) * 128],
                        rhs=tbl_sb[:100, :],
                        start=True,
                        stop=True,
                    )
                    if s == 3:
                        mm.then_inc(pout_rdy, 1)

        @block.sync
        def _(sync):
            sync.dma_start(out=iota_sb[:], in_=iota_d[:]).then_inc(in1, 16)
            sync.dma_start(out=tbl_sb[:], in_=tbl_d[:]).then_inc(in2, 16)
            for g in range(N_GROUPS):
                sync.wait_ge(wbA, 2 * (g + 1))
                sync.wait_ge(wbB, 2 * (g + 1))
                if g >= 2:
                    sync.wait_ge(wbd[g % 2], (g // 2) * 16)
                sync.dma_start(
                    out=out_flat[:, g * 1024 : (g + 1) * 1024],
                    in_=outb_sb[:, (g % 2) * 1024 : (g % 2) * 1024 + 1024],
                ).then_inc(wbd[g % 2], 16)
            sync.wait_ge(wbd0, (N_GROUPS // 2) * 16)
            sync.wait_ge(wbd1, (N_GROUPS // 2) * 16)

    return nc


def _prep_in_maps(atom_types, embedding):
    import ml_dtypes

    at = np.asarray(atom_types).astype(np.int32).reshape(-1)
    emb = np.asarray(embedding).astype(np.float32)

    # rank-remap: table2[x] = embedding[rank(x)] where rank(x) counts the
    # distinct values < x present anywhere in the batch (identity when all
    # NUM_TYPES values appear).
    present = np.zeros(NUM_TYPES, dtype=bool)
    present[at] = True
    rank = np.cumsum(present) - present
    table2 = emb[np.minimum(rank, NUM_TYPES - 1)].astype(np.float32)
    table2[~present] = 0.0

    iota_in = np.arange(128, dtype=np.float32).reshape(128, 1)
    tbl_in = np.zeros((128, 64), np.float32)
    tbl_in[:100] = table2

    in_maps = []
    for c in range(N_CORES):
        shard = at[c * ROWS_PER_CORE : (c + 1) * ROWS_PER_CORE]
        sp = np.concatenate(
            [shard, np.full(PAD_ROWS - ROWS_PER_CORE, shard[0], np.int32)]
        )
        # atom (p, t) = sp[p*T + t]; chunk k covers slots 4k..4k+3;
        # within-chunk position q = s*128 + p.
        grid = sp.reshape(128, T_SLOTS)  # [p, t]
        feed = grid.reshape(128, N_CHUNKS, 4).transpose(1, 2, 0)  # [k, s, p]
        in_maps.append(
            {
                "idx": feed.reshape(1, -1).astype(ml_dtypes.bfloat16),
                "iota": iota_in,
                "tbl": tbl_in.astype(ml_dtypes.bfloat16),
            }
        )
    return in_maps


def run(atom_types, embedding, trace=False):
    from concourse.bass_utils import run_bass_kernel_spmd

    if "nc" not in _CACHE:
        _CACHE["nc"] = _build_graph()
    nc = _CACHE["nc"]

    in_maps = _prep_in_maps(atom_types, embedding)
    res = run_bass_kernel_spmd(
        nc, in_maps, core_ids=list(range(N_CORES)), trace=trace
    )
    shards = [r["out"][:ROWS_PER_CORE] for r in res.results]
    full = np.concatenate(shards, axis=0).reshape(N_BATCH, ATOMS_PER_MOL, EMBED_DIM)
    return np.ascontiguousarray(full, dtype=np.float32), res


def kernel(atom_types, embedding):
    out, _ = run(atom_types, embedding, trace=False)
    return out


# revision 31
# speedup vs baseline: 2.4298x; 1.8171x over previous
"""AtomEmbedding kernel for 8 TRN2 NeuronCores.

Reference semantics: rank-remap of atom types through the sorted unique
values present in the batch, then embedding lookup:
    uniq = unique(atom_types)  (padded sorted)
    out[b, a] = embedding[searchsorted(uniq, atom_types[b, a])]

Device strategy (data-parallel over n_batch): the lookup is computed as a
one-hot matmul on the TensorEngine (a per-row DMA gather is descriptor-
generation-bound on the Q7 cores at ~7 ns/row — measured 542 us/core):

  per 2048-atom batch b (4 chunks of 512):
    POOL : SWDGE broadcast DMA replicating the batch's bf16 index rows
           from DRAM onto 100 SBUF partitions (0-stride AP)
  per 512-atom chunk k:
    DVE  : oh[v, q] = (rep == iota_v)        bf16 one-hot
    PE   : out[p, d] = oh[:, s*128:].T @ table_bf16[100, 64]
  per PSUM bank (2 chunks): ACT/DVE alternate [128,512] PSUM->SBUF copies
  per group (4 chunks): sync writes 4KB/partition to DRAM.

Raw-bass engine blocks with standalone wait_ge (the neuronxcc walrus in
this toolchain cannot encode multi-wait sync on one instruction).

Self-contained: shapes hardcoded, no sibling imports.
"""

import sys

if "/opt/trn_rl_repo" not in sys.path:
    sys.path.insert(0, "/opt/trn_rl_repo")

import numpy as np

N_BATCH = 9000
ATOMS_PER_MOL = 64
EMBED_DIM = 64
NUM_TYPES = 100
N_CORES = 8

ROWS_PER_CORE = N_BATCH * ATOMS_PER_MOL // N_CORES  # 72000
T_SLOTS = 576  # padded slots per partition
PAD_ROWS = 128 * T_SLOTS  # 73728
N_CHUNKS = T_SLOTS // 4  # 144 chunks of 512 atoms
N_BATCHES = N_CHUNKS // 4  # 36 replicate batches of 2048 atoms
N_GROUPS = N_CHUNKS // 4  # 36 writeback groups of 16 slots

_CACHE = {}


def _build_graph():
    import concourse.bass as bass
    import concourse.mybir as mybir

    f32 = mybir.dt.float32
    bf16 = mybir.dt.bfloat16
    AF = mybir.ActivationFunctionType
    OP = mybir.AluOpType

    nc = bass.Bass()

    idx_d = nc.declare_dram_parameter("idx", [1, N_CHUNKS * 512], bf16, isOutput=False)
    iota_d = nc.declare_dram_parameter("iota", [128, 1], f32, isOutput=False)
    tbl_d = nc.declare_dram_parameter("tbl", [128, 64], bf16, isOutput=False)
    out_d = nc.declare_dram_parameter("out", [PAD_ROWS, EMBED_DIM], f32, isOutput=True)

    with (
        nc.sbuf_tensor([128, 1], f32) as iota_sb,
        nc.sbuf_tensor([128, 64], bf16) as tbl_sb,
        nc.sbuf_tensor([100, 2 * 2048], bf16) as rep_sb,
        nc.sbuf_tensor([100, 3 * 512], bf16) as oh_sb,
        nc.sbuf_tensor([128, 2 * 1024], f32) as outb_sb,
        nc.psum_tensor([128, 512], f32) as pout0,
        nc.psum_tensor([128, 512], f32) as pout1,
        nc.semaphore("in1") as in1,
        nc.semaphore("in2") as in2,
        nc.semaphore("rd0") as rd0,
        nc.semaphore("rd1") as rd1,
        nc.semaphore("oh_rdy") as oh_rdy,
        nc.semaphore("pout_rdy") as pout_rdy,
        nc.semaphore("wbA") as wbA,
        nc.semaphore("wbB") as wbB,
        nc.semaphore("wbd0") as wbd0,
        nc.semaphore("wbd1") as wbd1,
        nc.Block() as block,
    ):
        pout = [pout0, pout1]
        rdma = [rd0, rd1]
        wbd = [wbd0, wbd1]
        iota_ap = iota_sb[:100, 0:1]
        out_flat = out_d[:].rearrange("(p t) d -> p (t d)", p=128)

        def repv(k):
            b = (k // 4) % 2
            return rep_sb[:, b * 2048 + (k % 4) * 512 : b * 2048 + (k % 4) * 512 + 512]

        def ohv(k):
            return oh_sb[:, (k % 3) * 512 : (k % 3) * 512 + 512]

        def bank_copy_done_wait(eng, b2):
            # wait until the [128,512] copy of psum bank index b2 is done
            if b2 % 2 == 0:
                eng.wait_ge(wbA, b2 // 2 + 1)
            else:
                eng.wait_ge(wbB, (b2 - 1) // 2 + 1)

        @block.gpsimd
        def _(g):
            for b in range(N_BATCHES):
                if b >= 2:
                    g.wait_ge(oh_rdy, 4 * (b - 1))  # DVE consumed batch b-2
                    g.wait_ge(rdma[b % 2], (b // 2) * 16)
                ap = idx_d[:]
                src = bass.AP(ap.tensor, b * 2048, [[0, 100], [1, 2048]])
                g.dma_start(
                    out=rep_sb[:, (b % 2) * 2048 : (b % 2) * 2048 + 2048], in_=src
                ).then_inc(rdma[b % 2], 16)

        @block.vector
        def _(dve):
            dve.wait_ge(in1, 16)
            for k in range(N_CHUNKS + 4):
                if k < N_CHUNKS:
                    b = k // 4
                    dve.wait_ge(rdma[b % 2], (b // 2 + 1) * 16)
                    if k >= 3:
                        dve.wait_ge(pout_rdy, k - 2)  # PE freed oh[k%3]
                    dve.tensor_scalar(
                        out=ohv(k),
                        in0=repv(k),
                        scalar1=iota_ap,
                        scalar2=None,
                        op0=OP.is_equal,
                    ).then_inc(oh_rdy, 1)
                kc = k - 4
                if kc >= 0 and kc % 4 == 2:  # odd bank b2 = kc//2
                    b2 = kc // 2
                    dve.wait_ge(pout_rdy, 2 * (b2 + 1))
                    g = b2 // 2
                    if g >= 2:
                        dve.wait_ge(wbd[g % 2], (g // 2) * 16)
                    dve.tensor_copy(
                        out=outb_sb[
                            :, (g % 2) * 1024 + 512 : (g % 2) * 1024 + 1024
                        ],
                        in_=pout[b2 % 2][:],
                    ).then_inc(wbB, 1)

        @block.scalar
        def _(act):
            for k in range(N_CHUNKS + 4):
                kc = k - 4
                if kc >= 0 and kc % 4 == 0:  # even bank b2 = kc//2
                    b2 = kc // 2
                    act.wait_ge(pout_rdy, 2 * (b2 + 1))
                    g = b2 // 2
                    if g >= 2:
                        act.wait_ge(wbd[g % 2], (g // 2) * 16)
                    act.activation(
                        out=outb_sb[:, (g % 2) * 1024 : (g % 2) * 1024 + 512],
                        in_=pout[b2 % 2][:],
                        func=AF.Copy,
                    ).then_inc(wbA, 1)

        @block.tensor
        def _(te):
            te.wait_ge(in2, 16)
            for k in range(N_CHUNKS):
                b2 = k // 2
                if b2 >= 2 and k % 2 == 0:
                    bank_copy_done_wait(te, b2 - 2)  # copy freed pout[b2%2]
                te.wait_ge(oh_rdy, k + 1)
                for s in range(4):
                    mm = te.matmul(
                        out=pout[b2 % 2][:, (k % 2) * 256 + s * 64 : (k % 2) * 256 + (s + 1) * 64],
                        lhsT=ohv(k)[:, s * 128 : (s + 1) * 128],
                        rhs=tbl_sb[:100, :],
                        start=True,
                        stop=True,
                    )
                    if s == 3:
                        mm.then_inc(pout_rdy, 1)

        @block.sync
        def _(sync):
            sync.dma_start(out=iota_sb[:], in_=iota_d[:]).then_inc(in1, 16)
            sync.dma_start(out=tbl_sb[:], in_=tbl_d[:]).then_inc(in2, 16)
            for g in range(N_GROUPS):
                sync.wait_ge(wbA, g + 1)
                sync.wait_ge(wbB, g + 1)
                if g >= 2:
                    sync.wait_ge(wbd[g % 2], (g // 2) * 16)
                sync.dma_start(
                    out=out_flat[:, g * 1024 : (g + 1) * 1024],
                    in_=outb_sb[:, (g % 2) * 1024 : (g % 2) * 1024 + 1024],
                ).then_inc(wbd[g % 2], 16)
            sync.wait_ge(wbd0, (N_GROUPS // 2) * 16)
            sync.wait_ge(wbd1, (N_GROUPS // 2) * 16)

    return nc


def _prep_in_maps(atom_types, embedding):
    import ml_dtypes

    at = np.asarray(atom_types).astype(np.int32).reshape(-1)
    emb = np.asarray(embedding).astype(np.float32)

    # rank-remap: table2[x] = embedding[rank(x)] where rank(x) counts the
    # distinct values < x present anywhere in the batch (identity when all
    # NUM_TYPES values appear).
    present = np.zeros(NUM_TYPES, dtype=bool)
    present[at] = True
    rank = np.cumsum(present) - present
    table2 = emb[np.minimum(rank, NUM_TYPES - 1)].astype(np.float32)
    table2[~present] = 0.0

    iota_in = np.arange(128, dtype=np.float32).reshape(128, 1)
    tbl_in = np.zeros((128, 64), np.float32)
    tbl_in[:100] = table2

    in_maps = []
    for c in range(N_CORES):
        shard = at[c * ROWS_PER_CORE : (c + 1) * ROWS_PER_CORE]
        sp = np.concatenate(
            [shard, np.full(PAD_ROWS - ROWS_PER_CORE, shard[0], np.int32)]
        )
        # atom (p, t) = sp[p*T + t]; chunk k covers slots 4k..4k+3;
        # within-chunk position q = s*128 + p.
        grid = sp.reshape(128, T_SLOTS)  # [p, t]
        feed = grid.reshape(128, N_CHUNKS, 4).transpose(1, 2, 0)  # [k, s, p]
        in_maps.append(
            {
                "idx": feed.reshape(1, -1).astype(ml_dtypes.bfloat16),
                "iota": iota_in,
                "tbl": tbl_in.astype(ml_dtypes.bfloat16),
            }
        )
    return in_maps


def run(atom_types, embedding, trace=False):
    from concourse.bass_utils import run_bass_kernel_spmd

    if "nc" not in _CACHE:
        _CACHE["nc"] = _build_graph()
    nc = _CACHE["nc"]

    in_maps = _prep_in_maps(atom_types, embedding)
    res = run_bass_kernel_spmd(
        nc, in_maps, core_ids=list(range(N_CORES)), trace=trace
    )
    shards = [r["out"][:ROWS_PER_CORE] for r in res.results]
    full = np.concatenate(shards, axis=0).reshape(N_BATCH, ATOMS_PER_MOL, EMBED_DIM)
    return np.ascontiguousarray(full, dtype=np.float32), res


def kernel(atom_types, embedding):
    out, _ = run(atom_types, embedding, trace=False)
    return out


# revision 32
# speedup vs baseline: 2.5851x; 1.0639x over previous
"""AtomEmbedding kernel for 8 TRN2 NeuronCores.

Reference semantics: rank-remap of atom types through the sorted unique
values present in the batch, then embedding lookup:
    uniq = unique(atom_types)  (padded sorted)
    out[b, a] = embedding[searchsorted(uniq, atom_types[b, a])]

Device strategy (data-parallel over n_batch): the lookup is computed as a
one-hot matmul on the TensorEngine (a per-row DMA gather is descriptor-
generation-bound on the Q7 cores at ~7 ns/row — measured 542 us/core):

  per 2048-atom batch b (4 chunks of 512):
    POOL : SWDGE broadcast DMA replicating the batch's bf16 index rows
           from DRAM onto 100 SBUF partitions (0-stride AP)
  per 512-atom chunk k:
    DVE  : oh[v, q] = (rep == iota_v)        bf16 one-hot
    PE   : out[p, d] = oh[:, s*128:].T @ table_bf16[100, 64]
  per PSUM bank (2 chunks): ACT copies [128,512] PSUM->SBUF
  per group (4 chunks): sync writes 4KB/partition to DRAM.

Raw-bass engine blocks with standalone wait_ge (the neuronxcc walrus in
this toolchain cannot encode multi-wait sync on one instruction).

Self-contained: shapes hardcoded, no sibling imports.
"""

import sys

if "/opt/trn_rl_repo" not in sys.path:
    sys.path.insert(0, "/opt/trn_rl_repo")

import numpy as np

N_BATCH = 9000
ATOMS_PER_MOL = 64
EMBED_DIM = 64
NUM_TYPES = 100
N_CORES = 8

ROWS_PER_CORE = N_BATCH * ATOMS_PER_MOL // N_CORES  # 72000
T_SLOTS = 576  # padded slots per partition
PAD_ROWS = 128 * T_SLOTS  # 73728
N_CHUNKS = T_SLOTS // 4  # 144 chunks of 512 atoms
N_BATCHES = N_CHUNKS // 4  # 36 replicate batches of 2048 atoms
N_GROUPS = N_CHUNKS // 4  # 36 writeback groups of 16 slots
N_BANKS = N_CHUNKS // 2  # 72 psum banks of 2 chunks

_CACHE = {}


def _build_graph():
    import concourse.bass as bass
    import concourse.mybir as mybir

    f32 = mybir.dt.float32
    bf16 = mybir.dt.bfloat16
    AF = mybir.ActivationFunctionType
    OP = mybir.AluOpType

    nc = bass.Bass()

    idx_d = nc.declare_dram_parameter("idx", [1, N_CHUNKS * 512], bf16, isOutput=False)
    iota_d = nc.declare_dram_parameter("iota", [128, 1], f32, isOutput=False)
    tbl_d = nc.declare_dram_parameter("tbl", [128, 64], bf16, isOutput=False)
    out_d = nc.declare_dram_parameter("out", [PAD_ROWS, EMBED_DIM], f32, isOutput=True)

    with (
        nc.sbuf_tensor([128, 1], f32) as iota_sb,
        nc.sbuf_tensor([128, 64], bf16) as tbl_sb,
        nc.sbuf_tensor([100, 3 * 2048], bf16) as rep_sb,
        nc.sbuf_tensor([100, 4 * 512], bf16) as oh_sb,
        nc.sbuf_tensor([128, 3 * 1024], f32) as outb_sb,
        nc.psum_tensor([128, 512], f32) as pout0,
        nc.psum_tensor([128, 512], f32) as pout1,
        nc.psum_tensor([128, 512], f32) as pout2,
        nc.psum_tensor([128, 512], f32) as pout3,
        nc.semaphore("in1") as in1,
        nc.semaphore("in2") as in2,
        nc.semaphore("rd0") as rd0,
        nc.semaphore("rd1") as rd1,
        nc.semaphore("rd2") as rd2,
        nc.semaphore("oh_rdy") as oh_rdy,
        nc.semaphore("pout_rdy") as pout_rdy,
        nc.semaphore("wbA") as wbA,
        nc.semaphore("wbd0") as wbd0,
        nc.semaphore("wbd1") as wbd1,
        nc.semaphore("wbd2") as wbd2,
        nc.Block() as block,
    ):
        pout = [pout0, pout1, pout2, pout3]
        rdma = [rd0, rd1, rd2]
        wbd = [wbd0, wbd1, wbd2]
        iota_ap = iota_sb[:100, 0:1]
        out_flat = out_d[:].rearrange("(p t) d -> p (t d)", p=128)

        def repv(k):
            b = (k // 4) % 3
            return rep_sb[:, b * 2048 + (k % 4) * 512 : b * 2048 + (k % 4) * 512 + 512]

        def ohv(k):
            return oh_sb[:, (k % 4) * 512 : (k % 4) * 512 + 512]

        @block.gpsimd
        def _(g):
            for b in range(N_BATCHES):
                if b >= 3:
                    g.wait_ge(oh_rdy, 4 * (b - 2))  # DVE consumed batch b-3
                    g.wait_ge(rdma[b % 3], (b // 3) * 16)
                ap = idx_d[:]
                src = bass.AP(ap.tensor, b * 2048, [[0, 100], [1, 2048]])
                g.dma_start(
                    out=rep_sb[:, (b % 3) * 2048 : (b % 3) * 2048 + 2048], in_=src
                ).then_inc(rdma[b % 3], 16)

        @block.vector
        def _(dve):
            dve.wait_ge(in1, 16)
            for k in range(N_CHUNKS):
                b = k // 4
                dve.wait_ge(rdma[b % 3], (b // 3 + 1) * 16)
                if k >= 4:
                    dve.wait_ge(pout_rdy, k - 3)  # PE freed oh[k%4]
                dve.tensor_scalar(
                    out=ohv(k),
                    in0=repv(k),
                    scalar1=iota_ap,
                    scalar2=None,
                    op0=OP.is_equal,
                ).then_inc(oh_rdy, 1)

        @block.scalar
        def _(act):
            for b2 in range(N_BANKS):
                act.wait_ge(pout_rdy, 2 * (b2 + 1))
                g = b2 // 2
                if g >= 3:
                    act.wait_ge(wbd[g % 3], (g // 3) * 16)
                act.activation(
                    out=outb_sb[
                        :,
                        (g % 3) * 1024
                        + (b2 % 2) * 512 : (g % 3) * 1024
                        + (b2 % 2) * 512
                        + 512,
                    ],
                    in_=pout[b2 % 4][:],
                    func=AF.Copy,
                ).then_inc(wbA, 1)

        @block.tensor
        def _(te):
            te.wait_ge(in2, 16)
            for k in range(N_CHUNKS):
                b2 = k // 2
                if b2 >= 4 and k % 2 == 0:
                    te.wait_ge(wbA, b2 - 3)  # ACT copied bank b2-4
                te.wait_ge(oh_rdy, k + 1)
                for s in range(4):
                    mm = te.matmul(
                        out=pout[b2 % 4][
                            :, (k % 2) * 256 + s * 64 : (k % 2) * 256 + (s + 1) * 64
                        ],
                        lhsT=ohv(k)[:, s * 128 : (s + 1) * 128],
                        rhs=tbl_sb[:100, :],
                        start=True,
                        stop=True,
                    )
                    if s == 3:
                        mm.then_inc(pout_rdy, 1)

        @block.sync
        def _(sync):
            sync.dma_start(out=iota_sb[:], in_=iota_d[:]).then_inc(in1, 16)
            sync.dma_start(out=tbl_sb[:], in_=tbl_d[:]).then_inc(in2, 16)
            for g in range(N_GROUPS):
                sync.wait_ge(wbA, 2 * (g + 1))
                if g >= 3:
                    sync.wait_ge(wbd[g % 3], (g // 3) * 16)
                sync.dma_start(
                    out=out_flat[:, g * 1024 : (g + 1) * 1024],
                    in_=outb_sb[:, (g % 3) * 1024 : (g % 3) * 1024 + 1024],
                ).then_inc(wbd[g % 3], 16)
            for i in range(3):
                sync.wait_ge(wbd[i], (N_GROUPS // 3) * 16)

    return nc


def _prep_in_maps(atom_types, embedding):
    import ml_dtypes

    at = np.asarray(atom_types).astype(np.int32).reshape(-1)
    emb = np.asarray(embedding).astype(np.float32)

    # rank-remap: table2[x] = embedding[rank(x)] where rank(x) counts the
    # distinct values < x present anywhere in the batch (identity when all
    # NUM_TYPES values appear).
    present = np.zeros(NUM_TYPES, dtype=bool)
    present[at] = True
    rank = np.cumsum(present) - present
    table2 = emb[np.minimum(rank, NUM_TYPES - 1)].astype(np.float32)
    table2[~present] = 0.0

    iota_in = np.arange(128, dtype=np.float32).reshape(128, 1)
    tbl_in = np.zeros((128, 64), np.float32)
    tbl_in[:100] = table2

    in_maps = []
    for c in range(N_CORES):
        shard = at[c * ROWS_PER_CORE : (c + 1) * ROWS_PER_CORE]
        sp = np.concatenate(
            [shard, np.full(PAD_ROWS - ROWS_PER_CORE, shard[0], np.int32)]
        )
        # atom (p, t) = sp[p*T + t]; chunk k covers slots 4k..4k+3;
        # within-chunk position q = s*128 + p.
        grid = sp.reshape(128, T_SLOTS)  # [p, t]
        feed = grid.reshape(128, N_CHUNKS, 4).transpose(1, 2, 0)  # [k, s, p]
        in_maps.append(
            {
                "idx": feed.reshape(1, -1).astype(ml_dtypes.bfloat16),
                "iota": iota_in,
                "tbl": tbl_in.astype(ml_dtypes.bfloat16),
            }
        )
    return in_maps


def run(atom_types, embedding, trace=False):
    from concourse.bass_utils import run_bass_kernel_spmd

    if "nc" not in _CACHE:
        _CACHE["nc"] = _build_graph()
    nc = _CACHE["nc"]

    in_maps = _prep_in_maps(atom_types, embedding)
    res = run_bass_kernel_spmd(
        nc, in_maps, core_ids=list(range(N_CORES)), trace=trace
    )
    shards = [r["out"][:ROWS_PER_CORE] for r in res.results]
    full = np.concatenate(shards, axis=0).reshape(N_BATCH, ATOMS_PER_MOL, EMBED_DIM)
    return np.ascontiguousarray(full, dtype=np.float32), res


def kernel(atom_types, embedding):
    out, _ = run(atom_types, embedding, trace=False)
    return out


# revision 39
# speedup vs baseline: 2.6861x; 1.0391x over previous
"""AtomEmbedding kernel for 8 TRN2 NeuronCores.

Reference semantics: rank-remap of atom types through the sorted unique
values present in the batch, then embedding lookup:
    uniq = unique(atom_types)  (padded sorted)
    out[b, a] = embedding[searchsorted(uniq, atom_types[b, a])]

Device strategy (data-parallel over n_batch): the lookup is computed as a
one-hot matmul on the TensorEngine (a per-row DMA gather is descriptor-
generation-bound on the Q7 cores at ~7 ns/row — measured 542 us/core):

  per 2048-atom batch b (4 chunks of 512):
    POOL : SWDGE broadcast DMA replicating the batch's bf16 index rows
           from DRAM onto 100 SBUF partitions (0-stride AP)
  per 512-atom chunk k:
    DVE  : oh[v, q] = (rep == iota_v)        bf16 one-hot
    PE   : out[p, d] = oh[:, s*128:].T @ table_bf16[100, 64]
  per PSUM bank (2 chunks): ACT copies [128,512] PSUM->SBUF
  per group (4 chunks): sync writes 4KB/partition to DRAM.

Raw-bass engine blocks with standalone wait_ge (the neuronxcc walrus in
this toolchain cannot encode multi-wait sync on one instruction).

Self-contained: shapes hardcoded, no sibling imports.
"""

import sys

if "/opt/trn_rl_repo" not in sys.path:
    sys.path.insert(0, "/opt/trn_rl_repo")

import numpy as np

N_BATCH = 9000
ATOMS_PER_MOL = 64
EMBED_DIM = 64
NUM_TYPES = 100
N_CORES = 8

ROWS_PER_CORE = N_BATCH * ATOMS_PER_MOL // N_CORES  # 72000
T_SLOTS = 576  # padded slots per partition
PAD_ROWS = 128 * T_SLOTS  # 73728
N_CHUNKS = T_SLOTS // 4  # 144 chunks of 512 atoms
N_BATCHES = N_CHUNKS // 4  # 36 replicate batches of 2048 atoms
N_GROUPS = N_CHUNKS // 4  # 36 writeback groups of 16 slots
N_BANKS = N_CHUNKS // 2  # 72 psum banks of 2 chunks

_CACHE = {}


def _build_graph():
    import concourse.bass as bass
    import concourse.mybir as mybir

    f32 = mybir.dt.float32
    bf16 = mybir.dt.bfloat16
    AF = mybir.ActivationFunctionType
    OP = mybir.AluOpType

    nc = bass.Bass()

    idx_d = nc.declare_dram_parameter("idx", [10, N_CHUNKS * 512], bf16, isOutput=False)
    iota_d = nc.declare_dram_parameter("iota", [128, 1], f32, isOutput=False)
    tbl_d = nc.declare_dram_parameter("tbl", [128, 64], bf16, isOutput=False)
    out_d = nc.declare_dram_parameter("out", [PAD_ROWS, EMBED_DIM], f32, isOutput=True)

    from contextlib import ExitStack

    with ExitStack() as stack:
        iota_sb = stack.enter_context(nc.sbuf_tensor("iota_sb", [128, 1], f32))
        tbl_sb = stack.enter_context(nc.sbuf_tensor("tbl_sb", [128, 64], bf16))
        rep_sb = stack.enter_context(nc.sbuf_tensor("rep_sb", [100, 6 * 2048], bf16))
        oh_sb = stack.enter_context(nc.sbuf_tensor("oh_sb", [100, 4 * 512], bf16))
        outb_sb = stack.enter_context(nc.sbuf_tensor("outb_sb", [128, 3 * 2048], f32))
        pout = [
            stack.enter_context(nc.psum_tensor(f"pout{_i}", [128, 512], f32)) for _i in range(6)
        ]
        in1 = stack.enter_context(nc.semaphore("in1"))
        in2 = stack.enter_context(nc.semaphore("in2"))
        rp = [stack.enter_context(nc.semaphore(f"rp{i}")) for i in range(2)]
        ra = [stack.enter_context(nc.semaphore(f"ra{i}")) for i in range(2)]
        oh_rdy = stack.enter_context(nc.semaphore("oh_rdy"))
        pout_rdy = stack.enter_context(nc.semaphore("pout_rdy"))
        wbA = stack.enter_context(nc.semaphore("wbA"))
        wbB = stack.enter_context(nc.semaphore("wbB"))
        wbd = [stack.enter_context(nc.semaphore(f"wbd{i}")) for i in range(3)]
        block = stack.enter_context(nc.Block())
        iota_ap = iota_sb[:100, 0:1]
        out_flat = out_d[:].rearrange("(p t) d -> p (t d)", p=128)

        def repv(k):
            b = (k // 4) % 6
            return rep_sb[:, b * 2048 + (k % 4) * 512 : b * 2048 + (k % 4) * 512 + 512]

        def rep_dst(b):
            return rep_sb[:, (b % 6) * 2048 : (b % 6) * 2048 + 2048]

        def rep_src(b):
            ap = idx_d[:]
            return bass.AP(
                ap.tensor, b * 2048, [[N_CHUNKS * 512, 10], [0, 10], [1, 2048]]
            )

        def rep_sem(b):
            return (rp if b % 2 == 0 else ra)[(b // 2) % 2]

        def rep_val(b):
            return ((b // 2) // 2 + 1) * 16

        def bank_free_wait(eng, b2):
            # wait until the copy of psum bank b2 is done (ACT: even, DVE: odd)
            if b2 % 2 == 0:
                eng.wait_ge(wbA, b2 // 2 + 1)
            else:
                eng.wait_ge(wbB, (b2 - 1) // 2 + 1)

        def ohv(k):
            return oh_sb[:, (k % 4) * 512 : (k % 4) * 512 + 512]

        @block.gpsimd
        def _(g):
            for b in range(0, N_BATCHES, 2):
                if b >= 6:
                    g.wait_ge(oh_rdy, 4 * (b - 5))  # DVE consumed batch b-6
                g.wait_ge(rep_sem(b), rep_val(b) - 16)
                g.dma_start(out=rep_dst(b), in_=rep_src(b)).then_inc(rep_sem(b), 16)

        @block.vector
        def _(dve):
            dve.wait_ge(in1, 16)
            for k in range(N_CHUNKS + 8):
                if k < N_CHUNKS:
                    b = k // 4
                    dve.wait_ge(rep_sem(b), rep_val(b))
                    if k >= 4:
                        dve.wait_ge(pout_rdy, k - 3)  # PE freed oh[k%4]
                    dve.tensor_scalar(
                        out=ohv(k),
                        in0=repv(k),
                        scalar1=iota_ap,
                        scalar2=None,
                        op0=OP.is_equal,
                    ).then_inc(oh_rdy, 1)
                if k >= 6 and k % 4 == 2 and (k - 4) // 2 < N_BANKS:
                    b2 = (k - 4) // 2  # odd banks
                    dve.wait_ge(pout_rdy, 2 * (b2 + 1))
                    g8 = b2 // 4
                    if g8 >= 3:
                        dve.wait_ge(wbd[g8 % 3], (g8 // 3) * 16)
                    dve.tensor_copy(
                        out=outb_sb[
                            :,
                            (g8 % 3) * 2048
                            + (b2 % 4) * 512 : (g8 % 3) * 2048
                            + (b2 % 4) * 512
                            + 512,
                        ],
                        in_=pout[b2 % 6][:],
                    ).then_inc(wbB, 1)

        @block.scalar
        def _(act):
            for k in range(N_CHUNKS + 8):
                if k % 8 == 0:
                    b = k // 4 + 1  # odd batches, issued one batch ahead
                    if b < N_BATCHES:
                        if b >= 6:
                            act.wait_ge(oh_rdy, 4 * (b - 5))
                        act.wait_ge(rep_sem(b), rep_val(b) - 16)
                        act.dma_start(out=rep_dst(b), in_=rep_src(b)).then_inc(
                            rep_sem(b), 16
                        )
                if k >= 4 and k % 4 == 0 and (k - 4) // 2 < N_BANKS:
                    b2 = (k - 4) // 2  # even banks
                    act.wait_ge(pout_rdy, 2 * (b2 + 1))
                    g8 = b2 // 4
                    if g8 >= 3:
                        act.wait_ge(wbd[g8 % 3], (g8 // 3) * 16)
                    act.activation(
                        out=outb_sb[
                            :,
                            (g8 % 3) * 2048
                            + (b2 % 4) * 512 : (g8 % 3) * 2048
                            + (b2 % 4) * 512
                            + 512,
                        ],
                        in_=pout[b2 % 6][:],
                        func=AF.Copy,
                    ).then_inc(wbA, 1)

        @block.tensor
        def _(te):
            te.wait_ge(in2, 16)
            for k in range(N_CHUNKS):
                b2 = k // 2
                if b2 >= 6 and k % 2 == 0:
                    bank_free_wait(te, b2 - 6)
                te.wait_ge(oh_rdy, k + 1)
                for s in range(4):
                    mm = te.matmul(
                        out=pout[b2 % 6][
                            :, (k % 2) * 256 + s * 64 : (k % 2) * 256 + (s + 1) * 64
                        ],
                        lhsT=ohv(k)[:, s * 128 : (s + 1) * 128],
                        rhs=tbl_sb[:100, :],
                        start=True,
                        stop=True,
                    )
                    if s == 3:
                        mm.then_inc(pout_rdy, 1)

        @block.sync
        def _(sync):
            sync.dma_start(out=iota_sb[:], in_=iota_d[:]).then_inc(in1, 16)
            sync.dma_start(out=tbl_sb[:], in_=tbl_d[:]).then_inc(in2, 16)
            for g in range(N_GROUPS // 2):
                sync.wait_ge(wbA, 2 * (g + 1))
                sync.wait_ge(wbB, 2 * (g + 1))
                if g >= 3:
                    sync.wait_ge(wbd[g % 3], (g // 3) * 16)
                sync.dma_start(
                    out=out_flat[:, g * 2048 : (g + 1) * 2048],
                    in_=outb_sb[:, (g % 3) * 2048 : (g % 3) * 2048 + 2048],
                ).then_inc(wbd[g % 3], 16)
            for i in range(3):
                sync.wait_ge(wbd[i], (N_GROUPS // 2 // 3) * 16)

    return nc


def _prep_in_maps(atom_types, embedding):
    import ml_dtypes

    at = np.asarray(atom_types).astype(np.int32).reshape(-1)
    emb = np.asarray(embedding).astype(np.float32)

    # rank-remap: table2[x] = embedding[rank(x)] where rank(x) counts the
    # distinct values < x present anywhere in the batch (identity when all
    # NUM_TYPES values appear).
    present = np.zeros(NUM_TYPES, dtype=bool)
    present[at] = True
    rank = np.cumsum(present) - present
    table2 = emb[np.minimum(rank, NUM_TYPES - 1)].astype(np.float32)
    table2[~present] = 0.0

    iota_in = np.arange(128, dtype=np.float32).reshape(128, 1)
    tbl_in = np.zeros((128, 64), np.float32)
    tbl_in[:100] = table2

    in_maps = []
    for c in range(N_CORES):
        shard = at[c * ROWS_PER_CORE : (c + 1) * ROWS_PER_CORE]
        sp = np.concatenate(
            [shard, np.full(PAD_ROWS - ROWS_PER_CORE, shard[0], np.int32)]
        )
        # atom (p, t) = sp[p*T + t]; chunk k covers slots 4k..4k+3;
        # within-chunk position q = s*128 + p.
        grid = sp.reshape(128, T_SLOTS)  # [p, t]
        feed = grid.reshape(128, N_CHUNKS, 4).transpose(1, 2, 0)  # [k, s, p]
        in_maps.append(
            {
                "idx": np.tile(feed.reshape(1, -1).astype(ml_dtypes.bfloat16), (10, 1)),
                "iota": iota_in,
                "tbl": tbl_in.astype(ml_dtypes.bfloat16),
            }
        )
    return in_maps


def run(atom_types, embedding, trace=False):
    from concourse.bass_utils import run_bass_kernel_spmd

    if "nc" not in _CACHE:
        _CACHE["nc"] = _build_graph()
    nc = _CACHE["nc"]

    in_maps = _prep_in_maps(atom_types, embedding)
    res = run_bass_kernel_spmd(
        nc, in_maps, core_ids=list(range(N_CORES)), trace=trace
    )
    shards = [r["out"][:ROWS_PER_CORE] for r in res.results]
    full = np.concatenate(shards, axis=0).reshape(N_BATCH, ATOMS_PER_MOL, EMBED_DIM)
    return np.ascontiguousarray(full, dtype=np.float32), res


def kernel(atom_types, embedding):
    out, _ = run(atom_types, embedding, trace=False)
    return out


# revision 40
# speedup vs baseline: 2.9246x; 1.0888x over previous
"""AtomEmbedding kernel for 8 TRN2 NeuronCores.

Reference semantics: rank-remap of atom types through the sorted unique
values present in the batch, then embedding lookup:
    uniq = unique(atom_types)  (padded sorted)
    out[b, a] = embedding[searchsorted(uniq, atom_types[b, a])]

Device strategy (data-parallel over n_batch): the lookup is computed as a
one-hot matmul on the TensorEngine (a per-row DMA gather is descriptor-
generation-bound on the Q7 cores at ~7 ns/row — measured 542 us/core):

  per 2048-atom batch b (4 chunks of 512):
    POOL : SWDGE broadcast DMA replicating the batch's bf16 index rows
           from DRAM onto 100 SBUF partitions (0-stride AP)
  per 512-atom chunk k:
    DVE  : oh[v, q] = (rep == iota_v)        bf16 one-hot
    PE   : out[p, d] = oh[:, s*128:].T @ table_bf16[100, 64]
  per PSUM bank (2 chunks): ACT copies [128,512] PSUM->SBUF
  per group (4 chunks): sync writes 4KB/partition to DRAM.

Raw-bass engine blocks with standalone wait_ge (the neuronxcc walrus in
this toolchain cannot encode multi-wait sync on one instruction).

Self-contained: shapes hardcoded, no sibling imports.
"""

import sys

if "/opt/trn_rl_repo" not in sys.path:
    sys.path.insert(0, "/opt/trn_rl_repo")

import numpy as np

N_BATCH = 9000
ATOMS_PER_MOL = 64
EMBED_DIM = 64
NUM_TYPES = 100
N_CORES = 8

ROWS_PER_CORE = N_BATCH * ATOMS_PER_MOL // N_CORES  # 72000
T_SLOTS = 576  # padded slots per partition
PAD_ROWS = 128 * T_SLOTS  # 73728
N_CHUNKS = T_SLOTS // 4  # 144 chunks of 512 atoms
N_BATCHES = N_CHUNKS // 4  # 36 replicate batches of 2048 atoms
N_GROUPS = N_CHUNKS // 4  # 36 writeback groups of 16 slots
N_BANKS = N_CHUNKS // 2  # 72 psum banks of 2 chunks

_CACHE = {}


def _build_graph():
    import concourse.bass as bass
    import concourse.mybir as mybir

    f32 = mybir.dt.float32
    bf16 = mybir.dt.bfloat16
    AF = mybir.ActivationFunctionType
    OP = mybir.AluOpType

    nc = bass.Bass()

    idx_d = nc.declare_dram_parameter("idx", [10, N_CHUNKS * 512], bf16, isOutput=False)
    iota_d = nc.declare_dram_parameter("iota", [128, 1], f32, isOutput=False)
    tbl_d = nc.declare_dram_parameter("tbl", [128, 64], bf16, isOutput=False)
    out_d = nc.declare_dram_parameter("out", [PAD_ROWS, EMBED_DIM], f32, isOutput=True)

    from contextlib import ExitStack

    with ExitStack() as stack:
        iota_sb = stack.enter_context(nc.sbuf_tensor("iota_sb", [128, 1], f32))
        tbl_sb = stack.enter_context(nc.sbuf_tensor("tbl_sb", [128, 64], bf16))
        rep_sb = stack.enter_context(nc.sbuf_tensor("rep_sb", [100, 6 * 2048], bf16))
        oh_sb = stack.enter_context(nc.sbuf_tensor("oh_sb", [100, 4 * 512], bf16))
        outb_sb = stack.enter_context(nc.sbuf_tensor("outb_sb", [128, 3 * 2048], f32))
        pout = [
            stack.enter_context(nc.psum_tensor(f"pout{_i}", [128, 2048], f32))
            for _i in range(2)
        ]
        in1 = stack.enter_context(nc.semaphore("in1"))
        in2 = stack.enter_context(nc.semaphore("in2"))
        rp = [stack.enter_context(nc.semaphore(f"rp{i}")) for i in range(2)]
        ra = [stack.enter_context(nc.semaphore(f"ra{i}")) for i in range(2)]
        oh_rdy = stack.enter_context(nc.semaphore("oh_rdy"))
        pout_rdy = stack.enter_context(nc.semaphore("pout_rdy"))
        wbA = stack.enter_context(nc.semaphore("wbA"))
        wbB = stack.enter_context(nc.semaphore("wbB"))
        wbd = [stack.enter_context(nc.semaphore(f"wbd{i}")) for i in range(3)]
        block = stack.enter_context(nc.Block())
        iota_ap = iota_sb[:100, 0:1]
        out_flat = out_d[:].rearrange("(p t) d -> p (t d)", p=128)

        def repv(k):
            b = (k // 4) % 6
            return rep_sb[:, b * 2048 + (k % 4) * 512 : b * 2048 + (k % 4) * 512 + 512]

        def rep_dst(b):
            return rep_sb[:, (b % 6) * 2048 : (b % 6) * 2048 + 2048]

        def rep_src(b):
            ap = idx_d[:]
            return bass.AP(
                ap.tensor, b * 2048, [[N_CHUNKS * 512, 10], [0, 10], [1, 2048]]
            )

        def rep_sem(b):
            return (rp if b % 2 == 0 else ra)[(b // 2) % 2]

        def rep_val(b):
            return ((b // 2) // 2 + 1) * 16

        def bank_free_wait(eng, b2):
            # wait until the copy of psum bank b2 is done (ACT: even, DVE: odd)
            if b2 % 2 == 0:
                eng.wait_ge(wbA, b2 // 2 + 1)
            else:
                eng.wait_ge(wbB, (b2 - 1) // 2 + 1)

        def ohv(k):
            return oh_sb[:, (k % 4) * 512 : (k % 4) * 512 + 512]

        @block.gpsimd
        def _(g):
            for b in range(0, N_BATCHES, 2):
                if b >= 6:
                    g.wait_ge(oh_rdy, 4 * (b - 5))  # DVE consumed batch b-6
                g.wait_ge(rep_sem(b), rep_val(b) - 16)
                g.dma_start(out=rep_dst(b), in_=rep_src(b)).then_inc(rep_sem(b), 16)

        @block.vector
        def _(dve):
            dve.wait_ge(in1, 16)
            for k in range(N_CHUNKS + 8):
                if k < N_CHUNKS:
                    b = k // 4
                    dve.wait_ge(rep_sem(b), rep_val(b))
                    if k >= 4:
                        dve.wait_ge(pout_rdy, k - 3)  # PE freed oh[k%4]
                    dve.tensor_scalar(
                        out=ohv(k),
                        in0=repv(k),
                        scalar1=iota_ap,
                        scalar2=None,
                        op0=OP.is_equal,
                    ).then_inc(oh_rdy, 1)


        @block.scalar
        def _(act):
            for k in range(N_CHUNKS + 8):
                if k % 8 == 0:
                    b = k // 4 + 1  # odd batches, issued one batch ahead
                    if b < N_BATCHES:
                        if b >= 6:
                            act.wait_ge(oh_rdy, 4 * (b - 5))
                        act.wait_ge(rep_sem(b), rep_val(b) - 16)
                        act.dma_start(out=rep_dst(b), in_=rep_src(b)).then_inc(
                            rep_sem(b), 16
                        )
                if k >= 12 and k % 8 == 4 and (k - 12) // 8 < N_CHUNKS // 8:
                    B = (k - 12) // 8  # bankset = 8 chunks = one write group
                    act.wait_ge(pout_rdy, 8 * (B + 1))
                    if B >= 3:
                        act.wait_ge(wbd[B % 3], (B // 3) * 16)
                    act.activation(
                        out=outb_sb[:, (B % 3) * 2048 : (B % 3) * 2048 + 2048],
                        in_=pout[B % 2][:],
                        func=AF.Copy,
                    ).then_inc(wbA, 1)

        @block.tensor
        def _(te):
            te.wait_ge(in2, 16)
            for k in range(N_CHUNKS):
                B = k // 8
                if B >= 2 and k % 8 == 0:
                    te.wait_ge(wbA, B - 1)  # ACT copied bankset B-2
                te.wait_ge(oh_rdy, k + 1)
                for s in range(4):
                    mm = te.matmul(
                        out=pout[B % 2][
                            :, (k % 8) * 256 + s * 64 : (k % 8) * 256 + (s + 1) * 64
                        ],
                        lhsT=ohv(k)[:, s * 128 : (s + 1) * 128],
                        rhs=tbl_sb[:100, :],
                        start=True,
                        stop=True,
                    )
                    if s == 3:
                        mm.then_inc(pout_rdy, 1)

        @block.sync
        def _(sync):
            sync.dma_start(out=iota_sb[:], in_=iota_d[:]).then_inc(in1, 16)
            sync.dma_start(out=tbl_sb[:], in_=tbl_d[:]).then_inc(in2, 16)
            for g in range(N_GROUPS // 2):
                sync.wait_ge(wbA, g + 1)
                if g >= 3:
                    sync.wait_ge(wbd[g % 3], (g // 3) * 16)
                sync.dma_start(
                    out=out_flat[:, g * 2048 : (g + 1) * 2048],
                    in_=outb_sb[:, (g % 3) * 2048 : (g % 3) * 2048 + 2048],
                ).then_inc(wbd[g % 3], 16)
            for i in range(3):
                sync.wait_ge(wbd[i], (N_GROUPS // 2 // 3) * 16)

    return nc


def _prep_in_maps(atom_types, embedding):
    import ml_dtypes

    at = np.asarray(atom_types).astype(np.int32).reshape(-1)
    emb = np.asarray(embedding).astype(np.float32)

    # rank-remap: table2[x] = embedding[rank(x)] where rank(x) counts the
    # distinct values < x present anywhere in the batch (identity when all
    # NUM_TYPES values appear).
    present = np.zeros(NUM_TYPES, dtype=bool)
    present[at] = True
    rank = np.cumsum(present) - present
    table2 = emb[np.minimum(rank, NUM_TYPES - 1)].astype(np.float32)
    table2[~present] = 0.0

    iota_in = np.arange(128, dtype=np.float32).reshape(128, 1)
    tbl_in = np.zeros((128, 64), np.float32)
    tbl_in[:100] = table2

    in_maps = []
    for c in range(N_CORES):
        shard = at[c * ROWS_PER_CORE : (c + 1) * ROWS_PER_CORE]
        sp = np.concatenate(
            [shard, np.full(PAD_ROWS - ROWS_PER_CORE, shard[0], np.int32)]
        )
        # atom (p, t) = sp[p*T + t]; chunk k covers slots 4k..4k+3;
        # within-chunk position q = s*128 + p.
        grid = sp.reshape(128, T_SLOTS)  # [p, t]
        feed = grid.reshape(128, N_CHUNKS, 4).transpose(1, 2, 0)  # [k, s, p]
        in_maps.append(
            {
                "idx": np.tile(feed.reshape(1, -1).astype(ml_dtypes.bfloat16), (10, 1)),
                "iota": iota_in,
                "tbl": tbl_in.astype(ml_dtypes.bfloat16),
            }
        )
    return in_maps


def run(atom_types, embedding, trace=False):
    from concourse.bass_utils import run_bass_kernel_spmd

    if "nc" not in _CACHE:
        _CACHE["nc"] = _build_graph()
    nc = _CACHE["nc"]

    in_maps = _prep_in_maps(atom_types, embedding)
    res = run_bass_kernel_spmd(
        nc, in_maps, core_ids=list(range(N_CORES)), trace=trace
    )
    shards = [r["out"][:ROWS_PER_CORE] for r in res.results]
    full = np.concatenate(shards, axis=0).reshape(N_BATCH, ATOMS_PER_MOL, EMBED_DIM)
    return np.ascontiguousarray(full, dtype=np.float32), res


def kernel(atom_types, embedding):
    out, _ = run(atom_types, embedding, trace=False)
    return out


# revision 41
# speedup vs baseline: 2.9520x; 1.0094x over previous
"""AtomEmbedding kernel for 8 TRN2 NeuronCores.

Reference semantics: rank-remap of atom types through the sorted unique
values present in the batch, then embedding lookup:
    uniq = unique(atom_types)  (padded sorted)
    out[b, a] = embedding[searchsorted(uniq, atom_types[b, a])]

Device strategy (data-parallel over n_batch): the lookup is computed as a
one-hot matmul on the TensorEngine (a per-row DMA gather is descriptor-
generation-bound on the Q7 cores at ~7 ns/row — measured 542 us/core):

  per 2048-atom batch b (4 chunks of 512):
    POOL : SWDGE broadcast DMA replicating the batch's bf16 index rows
           from DRAM onto 100 SBUF partitions (0-stride AP)
  per 512-atom chunk k:
    DVE  : oh[v, q] = (rep == iota_v)        bf16 one-hot
    PE   : out[p, d] = oh[:, s*128:].T @ table_bf16[100, 64]
  per PSUM bank (2 chunks): ACT copies [128,512] PSUM->SBUF
  per group (4 chunks): sync writes 4KB/partition to DRAM.

Raw-bass engine blocks with standalone wait_ge (the neuronxcc walrus in
this toolchain cannot encode multi-wait sync on one instruction).

Self-contained: shapes hardcoded, no sibling imports.
"""

import sys

if "/opt/trn_rl_repo" not in sys.path:
    sys.path.insert(0, "/opt/trn_rl_repo")

import numpy as np

N_BATCH = 9000
ATOMS_PER_MOL = 64
EMBED_DIM = 64
NUM_TYPES = 100
N_CORES = 8

ROWS_PER_CORE = N_BATCH * ATOMS_PER_MOL // N_CORES  # 72000
T_SLOTS = 576  # padded slots per partition
PAD_ROWS = 128 * T_SLOTS  # 73728
N_CHUNKS = T_SLOTS // 4  # 144 chunks of 512 atoms
N_BATCHES = N_CHUNKS // 4  # 36 replicate batches of 2048 atoms
N_GROUPS = N_CHUNKS // 4  # 36 writeback groups of 16 slots
N_BANKS = N_CHUNKS // 2  # 72 psum banks of 2 chunks

_CACHE = {}


def _build_graph():
    import concourse.bass as bass
    import concourse.mybir as mybir

    f32 = mybir.dt.float32
    bf16 = mybir.dt.bfloat16
    AF = mybir.ActivationFunctionType
    OP = mybir.AluOpType

    nc = bass.Bass()

    idx_d = nc.declare_dram_parameter("idx", [10, N_CHUNKS * 512], bf16, isOutput=False)
    iota_d = nc.declare_dram_parameter("iota", [128, 1], f32, isOutput=False)
    tbl_d = nc.declare_dram_parameter("tbl", [128, 64], bf16, isOutput=False)
    out_d = nc.declare_dram_parameter("out", [PAD_ROWS, EMBED_DIM], f32, isOutput=True)

    from contextlib import ExitStack

    with ExitStack() as stack:
        iota_sb = stack.enter_context(nc.sbuf_tensor("iota_sb", [128, 1], f32))
        tbl_sb = stack.enter_context(nc.sbuf_tensor("tbl_sb", [128, 64], bf16))
        rep_sb = stack.enter_context(nc.sbuf_tensor("rep_sb", [100, 8 * 2048], bf16))
        oh_sb = stack.enter_context(nc.sbuf_tensor("oh_sb", [100, 4 * 512], bf16))
        outb_sb = stack.enter_context(nc.sbuf_tensor("outb_sb", [128, 3 * 2048], f32))
        pout = [
            stack.enter_context(nc.psum_tensor(f"pout{_i}", [128, 2048], f32))
            for _i in range(2)
        ]
        in1 = stack.enter_context(nc.semaphore("in1"))
        in2 = stack.enter_context(nc.semaphore("in2"))
        rp = [stack.enter_context(nc.semaphore(f"rp{i}")) for i in range(2)]
        ra = [stack.enter_context(nc.semaphore(f"ra{i}")) for i in range(2)]
        oh_rdy = stack.enter_context(nc.semaphore("oh_rdy"))
        pout_rdy = stack.enter_context(nc.semaphore("pout_rdy"))
        wbA = stack.enter_context(nc.semaphore("wbA"))
        wbB = stack.enter_context(nc.semaphore("wbB"))
        wbd = [stack.enter_context(nc.semaphore(f"wbd{i}")) for i in range(3)]
        block = stack.enter_context(nc.Block())
        iota_ap = iota_sb[:100, 0:1]
        out_flat = out_d[:].rearrange("(p t) d -> p (t d)", p=128)

        def repv(k):
            b = (k // 4) % 8
            return rep_sb[:, b * 2048 + (k % 4) * 512 : b * 2048 + (k % 4) * 512 + 512]

        def rep_dst(b):
            return rep_sb[:, (b % 8) * 2048 : (b % 8) * 2048 + 2048]

        def rep_src(b):
            ap = idx_d[:]
            return bass.AP(
                ap.tensor, b * 2048, [[N_CHUNKS * 512, 10], [0, 10], [1, 2048]]
            )

        def rep_sem(b):
            return (rp if b % 2 == 0 else ra)[(b // 2) % 2]

        def rep_val(b):
            return ((b // 2) // 2 + 1) * 16

        def bank_free_wait(eng, b2):
            # wait until the copy of psum bank b2 is done (ACT: even, DVE: odd)
            if b2 % 2 == 0:
                eng.wait_ge(wbA, b2 // 2 + 1)
            else:
                eng.wait_ge(wbB, (b2 - 1) // 2 + 1)

        def ohv(k):
            return oh_sb[:, (k % 4) * 512 : (k % 4) * 512 + 512]

        @block.gpsimd
        def _(g):
            for b in range(0, N_BATCHES, 2):
                if b >= 8:
                    g.wait_ge(oh_rdy, 4 * (b - 7))  # DVE consumed batch b-8
                g.wait_ge(rep_sem(b), rep_val(b) - 16)
                g.dma_start(out=rep_dst(b), in_=rep_src(b)).then_inc(rep_sem(b), 16)

        @block.vector
        def _(dve):
            dve.wait_ge(in1, 16)
            for k in range(N_CHUNKS + 8):
                if k < N_CHUNKS:
                    b = k // 4
                    dve.wait_ge(rep_sem(b), rep_val(b))
                    if k >= 4:
                        dve.wait_ge(pout_rdy, k - 3)  # PE freed oh[k%4]
                    dve.tensor_scalar(
                        out=ohv(k),
                        in0=repv(k),
                        scalar1=iota_ap,
                        scalar2=None,
                        op0=OP.is_equal,
                    ).then_inc(oh_rdy, 1)


        @block.scalar
        def _(act):
            for k in range(N_CHUNKS + 8):
                if k == 0:
                    for b0 in (1, 3):
                        act.dma_start(out=rep_dst(b0), in_=rep_src(b0)).then_inc(
                            rep_sem(b0), 16
                        )
                if k % 8 == 0 and k > 0:
                    b = k // 4 + 3  # odd batches, issued three batches ahead
                    if b < N_BATCHES:
                        if b >= 8:
                            act.wait_ge(oh_rdy, 4 * (b - 7))
                        act.wait_ge(rep_sem(b), rep_val(b) - 16)
                        act.dma_start(out=rep_dst(b), in_=rep_src(b)).then_inc(
                            rep_sem(b), 16
                        )
                if k >= 12 and k % 8 == 4 and (k - 12) // 8 < N_CHUNKS // 8:
                    B = (k - 12) // 8  # bankset = 8 chunks = one write group
                    act.wait_ge(pout_rdy, 8 * (B + 1))
                    if B >= 3:
                        act.wait_ge(wbd[B % 3], (B // 3) * 16)
                    act.activation(
                        out=outb_sb[:, (B % 3) * 2048 : (B % 3) * 2048 + 2048],
                        in_=pout[B % 2][:],
                        func=AF.Copy,
                    ).then_inc(wbA, 1)

        @block.tensor
        def _(te):
            te.wait_ge(in2, 16)
            for k in range(N_CHUNKS):
                B = k // 8
                if B >= 2 and k % 8 == 0:
                    te.wait_ge(wbA, B - 1)  # ACT copied bankset B-2
                te.wait_ge(oh_rdy, k + 1)
                for s in range(4):
                    mm = te.matmul(
                        out=pout[B % 2][
                            :, (k % 8) * 256 + s * 64 : (k % 8) * 256 + (s + 1) * 64
                        ],
                        lhsT=ohv(k)[:, s * 128 : (s + 1) * 128],
                        rhs=tbl_sb[:100, :],
                        start=True,
                        stop=True,
                    )
                    if s == 3:
                        mm.then_inc(pout_rdy, 1)

        @block.sync
        def _(sync):
            sync.dma_start(out=iota_sb[:], in_=iota_d[:]).then_inc(in1, 16)
            sync.dma_start(out=tbl_sb[:], in_=tbl_d[:]).then_inc(in2, 16)
            for g in range(N_GROUPS // 2):
                sync.wait_ge(wbA, g + 1)
                if g >= 3:
                    sync.wait_ge(wbd[g % 3], (g // 3) * 16)
                sync.dma_start(
                    out=out_flat[:, g * 2048 : (g + 1) * 2048],
                    in_=outb_sb[:, (g % 3) * 2048 : (g % 3) * 2048 + 2048],
                ).then_inc(wbd[g % 3], 16)
            for i in range(3):
                sync.wait_ge(wbd[i], (N_GROUPS // 2 // 3) * 16)

    return nc


def _prep_in_maps(atom_types, embedding):
    import ml_dtypes

    at = np.asarray(atom_types).astype(np.int32).reshape(-1)
    emb = np.asarray(embedding).astype(np.float32)

    # rank-remap: table2[x] = embedding[rank(x)] where rank(x) counts the
    # distinct values < x present anywhere in the batch (identity when all
    # NUM_TYPES values appear).
    present = np.zeros(NUM_TYPES, dtype=bool)
    present[at] = True
    rank = np.cumsum(present) - present
    table2 = emb[np.minimum(rank, NUM_TYPES - 1)].astype(np.float32)
    table2[~present] = 0.0

    iota_in = np.arange(128, dtype=np.float32).reshape(128, 1)
    tbl_in = np.zeros((128, 64), np.float32)
    tbl_in[:100] = table2

    in_maps = []
    for c in range(N_CORES):
        shard = at[c * ROWS_PER_CORE : (c + 1) * ROWS_PER_CORE]
        sp = np.concatenate(
            [shard, np.full(PAD_ROWS - ROWS_PER_CORE, shard[0], np.int32)]
        )
        # atom (p, t) = sp[p*T + t]; chunk k covers slots 4k..4k+3;
        # within-chunk position q = s*128 + p.
        grid = sp.reshape(128, T_SLOTS)  # [p, t]
        feed = grid.reshape(128, N_CHUNKS, 4).transpose(1, 2, 0)  # [k, s, p]
        in_maps.append(
            {
                "idx": np.tile(feed.reshape(1, -1).astype(ml_dtypes.bfloat16), (10, 1)),
                "iota": iota_in,
                "tbl": tbl_in.astype(ml_dtypes.bfloat16),
            }
        )
    return in_maps


def run(atom_types, embedding, trace=False):
    from concourse.bass_utils import run_bass_kernel_spmd

    if "nc" not in _CACHE:
        _CACHE["nc"] = _build_graph()
    nc = _CACHE["nc"]

    in_maps = _prep_in_maps(atom_types, embedding)
    res = run_bass_kernel_spmd(
        nc, in_maps, core_ids=list(range(N_CORES)), trace=trace
    )
    shards = [r["out"][:ROWS_PER_CORE] for r in res.results]
    full = np.concatenate(shards, axis=0).reshape(N_BATCH, ATOMS_PER_MOL, EMBED_DIM)
    return np.ascontiguousarray(full, dtype=np.float32), res


def kernel(atom_types, embedding):
    out, _ = run(atom_types, embedding, trace=False)
    return out
